# revision 19
# baseline (speedup 1.0000x reference)
"""Trainium2 Bass kernel for causal multi-head attention.

Problem (hardcoded): x [2, 2048, 1024] f32, W [1024, 3072] f32, b [3072] f32.
qkv = x @ W + b; split into Q, K, V (16 heads, head_dim 64); causal attention
with softmax(scale * masked(QK^T)), scale = 1/sqrt(1024/3); out [2, 2048, 1024].

Sharding: 8 cores = 2 batches x 4 head-groups (4 heads each). No cross-core
communication. Each core computes its batch's qkv slice and attention for its
4 heads, producing out[b, :, g*256:(g+1)*256].

Device dataflow (per core):
  - host supplies xT (x[b].T, bf16 [1024, 2048]) and wg (W group slice,
    bf16 [1024, 768]) so all contractions have the e-dim on partitions.
  - Q^T, K^T computed as W^T @ x^T -> [f, t] layout (what QK^T wants).
  - V computed in natural [t, f] layout via lhsT = x^T tiles, with bias added
    by a K=1 outer-product matmul and a ones column appended -> PV matmul
    yields both O and the softmax row-sums in one pass.
  - S^T[k, q] for a head pair lands in one 2-bank PSUM tile (the two heads'
    matmuls use disjoint 64-row groups of the PE array and run concurrently);
    causal skips tiles above the diagonal and shrinks the q-range per k-tile.
  - P^T = exp(scale * S^T) in ONE activation instruction per (qc, kt) pair
    (no max subtraction: logits are O(+-3), safe in f32); diagonal blocks are
    then masked by multiplying P^T with a 0/1 triangle in bf16 (DVE 4x mode).
  - O[q, 65] accumulated over k-tiles with lhsT = P^T subtiles; one PSUM
    accumulation group per bank (start on first write, stop on last).
  - out[q, d] = O[q, 0:64] * (1 / O[q, 64]).
"""

from contextlib import ExitStack

import ml_dtypes
import numpy as np

import concourse.bacc as bacc
import concourse.mybir as mybir
import concourse.tile as tile
from concourse.bass_utils import run_bass_kernel_spmd

T = 2048
E = 1024
F = 768          # per-core qkv features (4 heads x 64 x 3)
FQK = 512        # Q cols [0,256) + K cols [256,512) in wg
FV = 256         # V cols [512,768) in wg
HD = 64
NH = 4           # heads per core
SCALE = float(np.sqrt(3.0) / 32.0)  # 1/sqrt(1024/3)

BF16 = mybir.dt.bfloat16
F32 = mybir.dt.float32
AF = mybir.ActivationFunctionType


def emit_body(nc, tc, pools, dram, rep):
    """One full per-core MHA computation."""
    const, big, work, outp, psA, psS, psO = pools
    xT_d, wg_d, bqk_d, bv_d, mask_d, out_d = dram

    # ---- persistent SBUF tensors for this rep ----
    xT = big.tile([128, 8, T], BF16, tag="xT", name=f"xT{rep}")
    w_sb = big.tile([128, 8, F], BF16, tag="w", name=f"w{rep}")
    qkT = big.tile([128, 4, T], BF16, tag="qkT", name=f"qkT{rep}")
    v_sb = big.tile([128, 16, NH, HD + 1], BF16, tag="v", name=f"v{rep}")
    out_sb = outp.tile([128, 16, FV], F32, tag="o", name=f"o{rep}")

    # W and the first xT t-chunk interleaved per e-tile so the first
    # projection chunk only waits for its own quarter of xT.
    for et in range(8):
        nc.sync.dma_start(out=w_sb[:, et, :], in_=wg_d[et * 128:(et + 1) * 128, :])
        nc.sync.dma_start(
            out=xT[:, et, 0:512], in_=xT_d[et * 128:(et + 1) * 128, 0:512]
        )
    for tcd in range(1, 4):
        for et in range(8):
            nc.sync.dma_start(
                out=xT[:, et, tcd * 512:(tcd + 1) * 512],
                in_=xT_d[et * 128:(et + 1) * 128, tcd * 512:(tcd + 1) * 512],
            )
    nc.vector.memset(v_sb[:, :, :, HD], 1.0)

    # projection chunk tc4 is emitted, then attention chunk qc=tc4 (which
    # consumes exactly projection chunks <= tc4), so projection matmuls fill
    # PE gaps while ACT works through the exps of the attention chunk.
    for tc4 in range(4):
        emit_proj_chunk(nc, pools, const, rep, tc4, xT, w_sb, qkT, v_sb,
                        ft_list=(0, 2))
        emit_attn_chunk(nc, pools, const, rep, 0, tc4, qkT, v_sb, out_sb)
        emit_proj_chunk(nc, pools, const, rep, tc4, xT, w_sb, qkT, v_sb,
                        ft_list=(1, 3), tt_list=())
        emit_attn_chunk(nc, pools, const, rep, 1, tc4, qkT, v_sb, out_sb)
        for qs in range(tc4 * 4, tc4 * 4 + 4):
            nc.sync.dma_start(
                out=out_d[qs * 128:(qs + 1) * 128, :], in_=out_sb[:, qs, :]
            )


def emit_proj_chunk(nc, pools, const, rep, tc4, xT, w_sb, qkT, v_sb,
                    ft_list=(0, 1, 2, 3), tt_list=None):
    _, big, work, outp, psA, psS, psO = pools
    cbqk = const["bqk_sb"]
    cbv = const["bv_sb"]
    cones = const["ones_sb"]
    if tt_list is None:
        tt_list = range(tc4 * 4, tc4 * 4 + 4)
    if True:
        ts = slice(tc4 * 512, (tc4 + 1) * 512)
        # head-pair 0 needs f-tiles 0 (Q) and 2 (K); pair 1 needs 1 and 3.
        # Emit pair-0's tiles and V first so attention hp=0 starts earlier.
        for ft in ft_list:
            ps = psA.tile([128, 512], F32, tag="mm", name=f"qkv{rep}_{tc4}_{ft}")
            for et in range(8):
                nc.tensor.matmul(
                    ps,
                    lhsT=w_sb[:, et, ft * 128:(ft + 1) * 128],
                    rhs=xT[:, et, ts],
                    start=(et == 0),
                    stop=(et == 7),
                )
            # bias add (per-partition) + cast to bf16, on DVE
            nc.vector.tensor_scalar_add(qkT[:, ft, ts], ps, cbqk[:, ft:ft + 1])
        for tt in tt_list:
            psv = psA.tile([128, FV], F32, tag="mm", name=f"v{rep}_{tt}")
            for et in range(8):
                nc.tensor.matmul(
                    psv,
                    lhsT=xT[:, et, tt * 128:(tt + 1) * 128],
                    rhs=w_sb[:, et, FQK:F],
                    start=(et == 0),
                    stop=False,
                )
            # bias via K=1 outer product: ones[t] x bv[f]
            nc.tensor.matmul(psv, lhsT=cones, rhs=cbv, start=False, stop=True)
            nc.vector.tensor_copy(
                v_sb[:, tt, :, 0:HD], psv.rearrange("p (h d) -> p h d", h=NH)
            )


def emit_attn_chunk(nc, pools, const, rep, hp, qc, qkT, v_sb, out_sb):
    _, big, work, outp, psA, psS, psO = pools
    tri01 = const["tri01_sb"]
    if True:
        if True:
            psOs = [
                psO.tile([128, 4, HD + 1], F32, tag="O", name=f"O{rep}_{hp}_{qc}_{i}")
                for i in range(2)
            ]
            for kt in range(4 * qc + 4):
                q0 = max(qc * 512, kt * 128)
                qw = qc * 512 + 512 - q0
                pss = psS.tile([128, 2, 512], F32, tag="S",
                               name=f"S{rep}_{hp}_{qc}_{kt}")
                # the two heads use disjoint 64-row groups -> HW overlaps them
                for hh in range(2):
                    rs = hh * 64
                    nc.tensor.matmul(
                        pss[:, hh, 0:qw],
                        lhsT=qkT[rs:rs + 64, 2 + hp, kt * 128:(kt + 1) * 128],
                        rhs=qkT[rs:rs + 64, hp, q0:qc * 512 + 512],
                        start=True,
                        stop=True,
                    )
                pt = work.tile([128, 2, 512], BF16, tag="PT",
                               name=f"PT{rep}_{hp}_{qc}_{kt}")
                nc.scalar.activation(
                    out=pt[:, :, 0:qw],
                    in_=pss[:, :, 0:qw],
                    func=AF.Exp,
                    scale=SCALE,
                )
                diag = kt >= 4 * qc
                if diag:  # causal mask: zero lower triangle of first 128 cols
                    for hh in range(2):
                        nc.vector.tensor_mul(
                            pt[:, hh, 0:128], pt[:, hh, 0:128], tri01
                        )
                for hh in range(2):
                    h = hp * 2 + hh
                    # one PSUM accumulation group per bank: start on the first
                    # matmul into the bank, stop on the last; per-element
                    # has_written bits handle first-write vs accumulate.
                    for qs in range(max(kt, 4 * qc), 4 * qc + 4):
                        nc.tensor.matmul(
                            psOs[hh][:, qs - 4 * qc, :],
                            lhsT=pt[:, hh, qs * 128 - q0:qs * 128 - q0 + 128],
                            rhs=v_sb[:, kt, h, :],
                            start=(kt == 0 and qs == 4 * qc),
                            stop=(kt == 4 * qc + 3 and qs == 4 * qc + 3),
                        )
            # epilogue: divide by row sums, write to out_sb
            for hh in range(2):
                h = hp * 2 + hh
                rsum = work.tile([128, 4, 1], F32, tag="rs",
                                 name=f"rs{rep}_{hp}_{qc}_{hh}")
                nc.vector.reciprocal(rsum, psOs[hh][:, :, HD:HD + 1])
                for ql in range(4):
                    nc.vector.tensor_scalar_mul(
                        out_sb[:, 4 * qc + ql, h * HD:(h + 1) * HD],
                        psOs[hh][:, ql, 0:HD],
                        rsum[:, ql, :],
                    )


def build_program(reps=1):
    nc = bacc.Bacc("TRN2", target_bir_lowering=False, debug=False, num_devices=8)
    xT_d = nc.dram_tensor("xT", [E, T], BF16, kind="ExternalInput").ap()
    wg_d = nc.dram_tensor("wg", [E, F], BF16, kind="ExternalInput").ap()
    bqk_d = nc.dram_tensor("bqk", [128, 4], F32, kind="ExternalInput").ap()
    bv_d = nc.dram_tensor("bv", [1, FV], BF16, kind="ExternalInput").ap()
    mask_d = nc.dram_tensor("tri01", [128, 128], BF16, kind="ExternalInput").ap()
    out_d = nc.dram_tensor("out", [T, FV], F32, kind="ExternalOutput").ap()

    with tile.TileContext(nc) as tcx, ExitStack() as ctx:
        constp = ctx.enter_context(tcx.tile_pool(name="constp", bufs=1))
        big = ctx.enter_context(tcx.tile_pool(name="big", bufs=1))
        work = ctx.enter_context(tcx.tile_pool(name="work", bufs=8))
        outp = ctx.enter_context(tcx.tile_pool(name="outp", bufs=1))
        psA = ctx.enter_context(tcx.tile_pool(name="psA", bufs=2, space="PSUM"))
        psS = ctx.enter_context(tcx.tile_pool(name="psS", bufs=2, space="PSUM"))
        psO = ctx.enter_context(tcx.tile_pool(name="psO", bufs=2, space="PSUM"))

        tri01_sb = constp.tile([128, 128], BF16)
        nc.sync.dma_start(out=tri01_sb, in_=mask_d)
        bqk_sb = constp.tile([128, 4], F32)
        nc.sync.dma_start(out=bqk_sb, in_=bqk_d)
        bv_sb = constp.tile([1, FV], BF16)
        nc.sync.dma_start(out=bv_sb, in_=bv_d)
        ones_sb = constp.tile([1, 128], BF16)
        nc.vector.memset(ones_sb, 1.0)
        const = {
            "tri01_sb": tri01_sb,
            "bqk_sb": bqk_sb,
            "bv_sb": bv_sb,
            "ones_sb": ones_sb,
        }

        pools = (const, big, work, outp, psA, psS, psO)
        dram = (xT_d, wg_d, bqk_d, bv_d, mask_d, out_d)
        for rep in range(reps):
            emit_body(nc, tcx, pools, dram, rep)

    nc.compile()
    return nc


def prep_core_inputs(x, W, b):
    """Host-side marshalling: slice per core, cast bf16, pre-transpose x."""
    tri01 = np.where(
        np.arange(128)[:, None] <= np.arange(128)[None, :], 1.0, 0.0
    ).astype(ml_dtypes.bfloat16)
    in_maps = []
    for c in range(8):
        bb, g = c // 4, c % 4
        cols = slice(g * 256, (g + 1) * 256)
        wg = np.concatenate(
            [W[:, cols], W[:, 1024:][:, cols], W[:, 2048:][:, cols]], axis=1
        )
        bq = b[cols]
        bk = b[1024:2048][cols]
        bv = b[2048:3072][cols]
        bqk = np.concatenate([bq, bk]).reshape(4, 128).T.copy()
        in_maps.append({
            "xT": np.ascontiguousarray(x[bb].T).astype(ml_dtypes.bfloat16),
            "wg": wg.astype(ml_dtypes.bfloat16),
            "bqk": bqk.astype(np.float32),
            "bv": bv.reshape(1, FV).astype(ml_dtypes.bfloat16),
            "tri01": tri01,
        })
    return in_maps


_nc_cache = {}


def get_program(reps=1):
    if reps not in _nc_cache:
        _nc_cache[reps] = build_program(reps)
    return _nc_cache[reps]


def kernel(x, W, b):
    x = np.asarray(x, dtype=np.float32)
    W = np.asarray(W, dtype=np.float32)
    b = np.asarray(b, dtype=np.float32)
    nc = get_program(1)
    in_maps = prep_core_inputs(x, W, b)
    res = run_bass_kernel_spmd(nc, in_maps, core_ids=list(range(8)))
    out = np.empty((2, T, 1024), dtype=np.float32)
    for c in range(8):
        bb, g = c // 4, c % 4
        out[bb, :, g * 256:(g + 1) * 256] = res.results[c]["out"]
    return out


# revision 28
# speedup vs baseline: 1.6669x; 1.6669x over previous
"""Trainium2 Bass kernel for causal multi-head attention.

Problem (hardcoded): x [2, 2048, 1024] f32, W [1024, 3072] f32, b [3072] f32.
qkv = x @ W + b; split into Q, K, V (16 heads, head_dim 64); causal attention
with softmax(scale * masked(QK^T)), scale = 1/sqrt(1024/3); out [2, 2048, 1024].

Sharding: 8 cores = 2 batches x 4 head-groups (4 heads each). No cross-core
communication. Each core computes its batch's qkv slice and attention for its
4 heads, producing out[b, :, g*256:(g+1)*256].

Device dataflow (per core):
  - host supplies xT (x[b].T, bf16 [1024, 2048]) and wg (W group slice,
    bf16 [1024, 768]) so all contractions have the e-dim on partitions.
  - Q^T, K^T computed as W^T @ x^T -> [f, t] layout (what QK^T wants).
  - V computed in natural [t, f] layout via lhsT = x^T tiles, with bias added
    by a K=1 outer-product matmul and a ones column appended -> PV matmul
    yields both O and the softmax row-sums in one pass.
  - S^T[k, q] for a head pair lands in one 2-bank PSUM tile (the two heads'
    matmuls use disjoint 64-row groups of the PE array and run concurrently);
    causal skips tiles above the diagonal and shrinks the q-range per k-tile.
  - P^T = exp(scale * S^T) in ONE activation instruction per (qc, kt) pair
    (no max subtraction: logits are O(+-3), safe in f32); diagonal blocks are
    then masked by multiplying P^T with a 0/1 triangle in bf16 (DVE 4x mode).
  - O[q, 65] accumulated over k-tiles with lhsT = P^T subtiles; one PSUM
    accumulation group per bank (start on first write, stop on last).
  - out[q, d] = O[q, 0:64] * (1 / O[q, 64]).
"""

from contextlib import ExitStack

import ml_dtypes
import numpy as np

import concourse.bacc as bacc
import concourse.mybir as mybir
import concourse.tile as tile
from concourse.bass_utils import run_bass_kernel_spmd

T = 2048
E = 1024
F = 768          # per-core qkv features (4 heads x 64 x 3)
FQK = 512        # Q cols [0,256) + K cols [256,512) in wg
FV = 256         # V cols [512,768) in wg
HD = 64
NH = 4           # heads per core
SCALE = float(np.sqrt(3.0) / 32.0)  # 1/sqrt(1024/3)

BF16 = mybir.dt.bfloat16
F32 = mybir.dt.float32
AF = mybir.ActivationFunctionType


def emit_body(nc, tc, pools, dram, rep):
    """One full per-core MHA computation."""
    const, big, work, outp, psA, psS, psO = pools
    xT_d, wg_d, bqk_d, bv_d, mask_d, out_d = dram

    # ---- persistent SBUF tensors for this rep ----
    xT = big.tile([128, 8, T], BF16, tag="xT", name=f"xT{rep}")
    w_sb = big.tile([128, 8, F], BF16, tag="w", name=f"w{rep}")
    qkT = big.tile([128, 4, T], BF16, tag="qkT", name=f"qkT{rep}")
    v_sb = big.tile([128, 16, NH, HD + 1], BF16, tag="v", name=f"v{rep}")
    out_sb = outp.tile([128, 16, FV], F32, tag="o", name=f"o{rep}")

    # W and the first xT t-chunk interleaved per e-tile so the first
    # projection chunk only waits for its own quarter of xT.
    # W on the SWDGE (gpsimd) queue, xT on HWDGE -> the two streams start
    # in parallel and the first projection matmul's inputs land sooner.
    for et in range(8):
        nc.gpsimd.dma_start(out=w_sb[:, et, :], in_=wg_d[et * 128:(et + 1) * 128, :])
        nc.sync.dma_start(
            out=xT[:, et, 0:512], in_=xT_d[et * 128:(et + 1) * 128, 0:512]
        )
    for tcd in range(1, 4):
        for et in range(8):
            nc.sync.dma_start(
                out=xT[:, et, tcd * 512:(tcd + 1) * 512],
                in_=xT_d[et * 128:(et + 1) * 128, tcd * 512:(tcd + 1) * 512],
            )
    nc.vector.memset(v_sb[:, :, :, HD], 1.0)

    # projection chunk tc4 is emitted, then attention chunk qc=tc4 (which
    # consumes exactly projection chunks <= tc4), so projection matmuls fill
    # PE gaps while ACT works through the exps of the attention chunk.
    # NOTE: emission order IS dataflow order — every tile must be emitted
    # before any consumer (Tile deps only point backward in program order).
    for tc4 in range(4):
        # pair-0's Q/K f-tiles (0, 2) and the chunk's V tiles first so the
        # first S-matmuls + exps of attention pair 0 start ASAP.
        emit_proj_chunk(nc, pools, const, rep, tc4, xT, w_sb, qkT, v_sb,
                        ft_list=(0, 2))
        emit_attn_chunk(nc, pools, const, rep, 0, tc4, qkT, v_sb, out_sb)
        emit_proj_chunk(nc, pools, const, rep, tc4, xT, w_sb, qkT, v_sb,
                        ft_list=(1, 3), tt_list=())
        # head-pair 0's output columns (0:128) can leave while pair 1 computes
        for qs in range(tc4 * 4, tc4 * 4 + 4):
            nc.sync.dma_start(
                out=out_d[qs * 128:(qs + 1) * 128, 0:128],
                in_=out_sb[:, qs, 0:128],
            )
        emit_attn_chunk(nc, pools, const, rep, 1, tc4, qkT, v_sb, out_sb)
        for qs in range(tc4 * 4, tc4 * 4 + 4):
            nc.sync.dma_start(
                out=out_d[qs * 128:(qs + 1) * 128, 128:256],
                in_=out_sb[:, qs, 128:256],
            )


def emit_proj_chunk(nc, pools, const, rep, tc4, xT, w_sb, qkT, v_sb,
                    ft_list=(0, 1, 2, 3), tt_list=None):
    _, big, work, outp, psA, psS, psO = pools
    cbqk = const["bqk_sb"]
    cbv = const["bv_sb"]
    cones = const["ones_sb"]
    if tt_list is None:
        tt_list = range(tc4 * 4, tc4 * 4 + 4)
    if True:
        ts = slice(tc4 * 512, (tc4 + 1) * 512)
        # head-pair 0 needs f-tiles 0 (Q) and 2 (K); pair 1 needs 1 and 3.
        # Emit pair-0's tiles and V first so attention hp=0 starts earlier.
        for ft in ft_list:
            ps = psA.tile([128, 512], F32, tag="mm", name=f"qkv{rep}_{tc4}_{ft}")
            for et in range(8):
                nc.tensor.matmul(
                    ps,
                    lhsT=w_sb[:, et, ft * 128:(ft + 1) * 128],
                    rhs=xT[:, et, ts],
                    start=(et == 0),
                    stop=(et == 7),
                )
            # bias add (per-partition) + cast to bf16, on DVE
            nc.vector.tensor_scalar_add(qkT[:, ft, ts], ps, cbqk[:, ft:ft + 1])
        for tt in tt_list:
            psv = psA.tile([128, FV], F32, tag="mm", name=f"v{rep}_{tt}")
            for et in range(8):
                nc.tensor.matmul(
                    psv,
                    lhsT=xT[:, et, tt * 128:(tt + 1) * 128],
                    rhs=w_sb[:, et, FQK:F],
                    start=(et == 0),
                    stop=False,
                )
            # bias via K=1 outer product: ones[t] x bv[f]
            nc.tensor.matmul(psv, lhsT=cones, rhs=cbv, start=False, stop=True)
            nc.vector.tensor_copy(
                v_sb[:, tt, :, 0:HD], psv.rearrange("p (h d) -> p h d", h=NH)
            )


def emit_attn_chunk(nc, pools, const, rep, hp, qc, qkT, v_sb, out_sb):
    _, big, work, outp, psA, psS, psO = pools
    tri01 = const["tri01_sb"]
    if True:
        if True:
            psOs = [
                psO.tile([128, 4, HD + 1], F32, tag="O", name=f"O{rep}_{hp}_{qc}_{i}")
                for i in range(2)
            ]
            for kt in range(4 * qc + 4):
                q0 = max(qc * 512, kt * 128)
                qw = qc * 512 + 512 - q0
                pss = psS.tile([128, 2, 512], F32, tag="S",
                               name=f"S{rep}_{hp}_{qc}_{kt}")
                # the two heads use disjoint 64-row groups -> HW overlaps them
                for hh in range(2):
                    rs = hh * 64
                    nc.tensor.matmul(
                        pss[:, hh, 0:qw],
                        lhsT=qkT[rs:rs + 64, 2 + hp, kt * 128:(kt + 1) * 128],
                        rhs=qkT[rs:rs + 64, hp, q0:qc * 512 + 512],
                        start=True,
                        stop=True,
                    )
                pt = work.tile([128, 2, 512], BF16, tag="PT",
                               name=f"PT{rep}_{hp}_{qc}_{kt}")
                nc.scalar.activation(
                    out=pt[:, :, 0:qw],
                    in_=pss[:, :, 0:qw],
                    func=AF.Exp,
                    scale=SCALE,
                )
                diag = kt >= 4 * qc
                if diag:  # causal mask: zero lower triangle of first 128 cols
                    for hh in range(2):
                        nc.vector.tensor_mul(
                            pt[:, hh, 0:128], pt[:, hh, 0:128], tri01
                        )
                for hh in range(2):
                    h = hp * 2 + hh
                    # one PSUM accumulation group per bank: start on the first
                    # matmul into the bank, stop on the last; per-element
                    # has_written bits handle first-write vs accumulate.
                    for qs in range(max(kt, 4 * qc), 4 * qc + 4):
                        nc.tensor.matmul(
                            psOs[hh][:, qs - 4 * qc, :],
                            lhsT=pt[:, hh, qs * 128 - q0:qs * 128 - q0 + 128],
                            rhs=v_sb[:, kt, h, :],
                            start=(kt == 0 and qs == 4 * qc),
                            stop=(kt == 4 * qc + 3 and qs == 4 * qc + 3),
                        )
            # epilogue: divide by row sums, write to out_sb
            for hh in range(2):
                h = hp * 2 + hh
                rsum = work.tile([128, 4, 1], F32, tag="rs",
                                 name=f"rs{rep}_{hp}_{qc}_{hh}")
                nc.vector.reciprocal(rsum, psOs[hh][:, :, HD:HD + 1])
                for ql in range(4):
                    nc.vector.tensor_scalar_mul(
                        out_sb[:, 4 * qc + ql, h * HD:(h + 1) * HD],
                        psOs[hh][:, ql, 0:HD],
                        rsum[:, ql, :],
                    )


def build_program(reps=1):
    nc = bacc.Bacc("TRN2", target_bir_lowering=False, debug=False, num_devices=8)
    xT_d = nc.dram_tensor("xT", [E, T], BF16, kind="ExternalInput").ap()
    wg_d = nc.dram_tensor("wg", [E, F], BF16, kind="ExternalInput").ap()
    bqk_d = nc.dram_tensor("bqk", [128, 4], F32, kind="ExternalInput").ap()
    bv_d = nc.dram_tensor("bv", [1, FV], BF16, kind="ExternalInput").ap()
    mask_d = nc.dram_tensor("tri01", [128, 128], BF16, kind="ExternalInput").ap()
    out_d = nc.dram_tensor("out", [T, FV], F32, kind="ExternalOutput").ap()

    with tile.TileContext(nc) as tcx, ExitStack() as ctx:
        constp = ctx.enter_context(tcx.tile_pool(name="constp", bufs=1))
        big = ctx.enter_context(tcx.tile_pool(name="big", bufs=1))
        work = ctx.enter_context(tcx.tile_pool(name="work", bufs=8))
        outp = ctx.enter_context(tcx.tile_pool(name="outp", bufs=1))
        psA = ctx.enter_context(tcx.tile_pool(name="psA", bufs=2, space="PSUM"))
        psS = ctx.enter_context(tcx.tile_pool(name="psS", bufs=2, space="PSUM"))
        psO = ctx.enter_context(tcx.tile_pool(name="psO", bufs=2, space="PSUM"))

        tri01_sb = constp.tile([128, 128], BF16)
        nc.sync.dma_start(out=tri01_sb, in_=mask_d)
        bqk_sb = constp.tile([128, 4], F32)
        nc.sync.dma_start(out=bqk_sb, in_=bqk_d)
        bv_sb = constp.tile([1, FV], BF16)
        nc.sync.dma_start(out=bv_sb, in_=bv_d)
        ones_sb = constp.tile([1, 128], BF16)
        nc.vector.memset(ones_sb, 1.0)
        const = {
            "tri01_sb": tri01_sb,
            "bqk_sb": bqk_sb,
            "bv_sb": bv_sb,
            "ones_sb": ones_sb,
        }

        pools = (const, big, work, outp, psA, psS, psO)
        dram = (xT_d, wg_d, bqk_d, bv_d, mask_d, out_d)
        for rep in range(reps):
            emit_body(nc, tcx, pools, dram, rep)

    nc.compile()
    return nc


def prep_core_inputs(x, W, b):
    """Host-side marshalling: slice per core, cast bf16, pre-transpose x."""
    tri01 = np.where(
        np.arange(128)[:, None] <= np.arange(128)[None, :], 1.0, 0.0
    ).astype(ml_dtypes.bfloat16)
    in_maps = []
    for c in range(8):
        bb, g = c // 4, c % 4
        cols = slice(g * 256, (g + 1) * 256)
        wg = np.concatenate(
            [W[:, cols], W[:, 1024:][:, cols], W[:, 2048:][:, cols]], axis=1
        )
        bq = b[cols]
        bk = b[1024:2048][cols]
        bv = b[2048:3072][cols]
        bqk = np.concatenate([bq, bk]).reshape(4, 128).T.copy()
        in_maps.append({
            "xT": np.ascontiguousarray(x[bb].T).astype(ml_dtypes.bfloat16),
            "wg": wg.astype(ml_dtypes.bfloat16),
            "bqk": bqk.astype(np.float32),
            "bv": bv.reshape(1, FV).astype(ml_dtypes.bfloat16),
            "tri01": tri01,
        })
    return in_maps


_nc_cache = {}


def get_program(reps=1):
    if reps not in _nc_cache:
        _nc_cache[reps] = build_program(reps)
    return _nc_cache[reps]


def kernel(x, W, b):
    x = np.asarray(x, dtype=np.float32)
    W = np.asarray(W, dtype=np.float32)
    b = np.asarray(b, dtype=np.float32)
    nc = get_program(1)
    in_maps = prep_core_inputs(x, W, b)
    res = run_bass_kernel_spmd(nc, in_maps, core_ids=list(range(8)))
    out = np.empty((2, T, 1024), dtype=np.float32)
    for c in range(8):
        bb, g = c // 4, c % 4
        out[bb, :, g * 256:(g + 1) * 256] = res.results[c]["out"]
    return out


# revision 36
# speedup vs baseline: 2.2649x; 1.3587x over previous
"""Trainium2 Bass kernel for causal multi-head attention.

Problem (hardcoded): x [2, 2048, 1024] f32, W [1024, 3072] f32, b [3072] f32.
qkv = x @ W + b; split into Q, K, V (16 heads, head_dim 64); causal attention
with softmax(scale * masked(QK^T)), scale = 1/sqrt(1024/3); out [2, 2048, 1024].

Sharding: 8 cores = 2 batches x 4 head-groups (4 heads each). No cross-core
communication. Each core computes its batch's qkv slice and attention for its
4 heads, producing out[b, :, g*256:(g+1)*256].

Device dataflow (per core):
  - host supplies xT (x[b].T, bf16 [1024, 2048]) and wg (W group slice,
    bf16 [1024, 768]) so all contractions have the e-dim on partitions.
  - Q^T, K^T computed as W^T @ x^T -> [f, t] layout (what QK^T wants).
  - V computed in natural [t, f] layout via lhsT = x^T tiles, with bias added
    by a K=1 outer-product matmul and a ones column appended -> PV matmul
    yields both O and the softmax row-sums in one pass.
  - S^T[k, q] for a head pair lands in one 2-bank PSUM tile (the two heads'
    matmuls use disjoint 64-row groups of the PE array and run concurrently);
    causal skips tiles above the diagonal and shrinks the q-range per k-tile.
  - P^T = exp(scale * S^T) in ONE activation instruction per (qc, kt) pair
    (no max subtraction: logits are O(+-3), safe in f32); diagonal blocks are
    then masked by multiplying P^T with a 0/1 triangle in bf16 (DVE 4x mode).
  - O[q, 65] accumulated over k-tiles with lhsT = P^T subtiles; one PSUM
    accumulation group per bank (start on first write, stop on last).
  - out[q, d] = O[q, 0:64] * (1 / O[q, 64]).
"""

from contextlib import ExitStack

import ml_dtypes
import numpy as np

import concourse.bacc as bacc
import concourse.bass as bass
import concourse.mybir as mybir
import concourse.tile as tile
from concourse.bass_utils import run_bass_kernel_spmd

T = 2048
E = 1024
F = 768          # per-core qkv features (4 heads x 64 x 3)
FQK = 512        # Q cols [0,256) + K cols [256,512) in wg
FV = 256         # V cols [512,768) in wg
HD = 64
NH = 4           # heads per core
SCALE = float(np.sqrt(3.0) / 32.0)  # 1/sqrt(1024/3)

BF16 = mybir.dt.bfloat16
F32 = mybir.dt.float32
AF = mybir.ActivationFunctionType


def emit_body(nc, tc, pools, dram, rep):
    """One full per-core MHA computation."""
    const, big, work, outp, psA, psS, psO = pools
    xT_d, wg_d, bqk_d, bv_d, mask_d, out_d = dram

    # ---- persistent SBUF tensors for this rep ----
    xT = big.tile([128, 8, T], BF16, tag="xT", name=f"xT{rep}")
    w_sb = big.tile([128, 8, F], BF16, tag="w", name=f"w{rep}")
    qkT = big.tile([128, 4, T], BF16, tag="qkT", name=f"qkT{rep}")
    v_sb = big.tile([128, 16, NH, HD + 1], BF16, tag="v", name=f"v{rep}")
    out_sb = outp.tile([128, 16, FV], F32, tag="o", name=f"o{rep}")

    # W on the SWDGE (gpsimd) queue, xT on HWDGE -> the two streams start
    # in parallel and the first projection matmul's inputs land sooner.
    for et in range(8):
        nc.gpsimd.dma_start(out=w_sb[:, et, :], in_=wg_d[et * 128:(et + 1) * 128, :])
        nc.sync.dma_start(
            out=xT[:, et, 0:512], in_=xT_d[et * 128:(et + 1) * 128, 0:512]
        )
    for tcd in range(1, 4):
        for et in range(8):
            nc.sync.dma_start(
                out=xT[:, et, tcd * 512:(tcd + 1) * 512],
                in_=xT_d[et * 128:(et + 1) * 128, tcd * 512:(tcd + 1) * 512],
            )
    nc.vector.memset(v_sb[:, :, :, HD], 1.0)

    # Emission schedule. Projection chunk t is interleaved with attention
    # chunk q=t (pair-0's Q/K f-tiles and V first so pair-0's S-matmuls +
    # exps start ASAP; pair-1's f-tiles between the two attention pairs).
    # The ACT-heavy last q-chunk is SPLIT: its first 8 k-tiles run early
    # (right after its Q projection, filling ACT's mid-kernel idle), the
    # partial O is parked in SBUF, and the chunk resumes at the end.
    # NOTE: emission order IS dataflow order — every tile must be emitted
    # before any consumer (Tile deps only point backward in program order).
    def out_dma(qc, hp):
        c0 = hp * 128
        for qs in range(qc * 4, qc * 4 + 4):
            nc.sync.dma_start(
                out=out_d[qs * 128:(qs + 1) * 128, c0:c0 + 128],
                in_=out_sb[:, qs, c0:c0 + 128],
            )

    parks = [
        outp.tile([128, 2, 4, HD + 1], F32, tag=f"park{hp}", name=f"pk{rep}_{hp}")
        for hp in range(2)
    ]

    def chunk(tc4, kt_lo=0, kt_hi=None, park_to=None, park_from=None):
        emit_proj_chunk(nc, pools, const, rep, tc4, xT, w_sb, qkT, v_sb,
                        ft_list=(0, 2))
        emit_attn_chunk(nc, pools, const, rep, 0, tc4, qkT, v_sb, out_sb,
                        kt_lo, kt_hi, parks[0] if park_to else None,
                        parks[0] if park_from else None)
        emit_proj_chunk(nc, pools, const, rep, tc4, xT, w_sb, qkT, v_sb,
                        ft_list=(1, 3), tt_list=())
        if not park_to:
            out_dma(tc4, 0)
        emit_attn_chunk(nc, pools, const, rep, 1, tc4, qkT, v_sb, out_sb,
                        kt_lo, kt_hi, parks[1] if park_to else None,
                        parks[1] if park_from else None)
        if not park_to:
            out_dma(tc4, 1)

    for tc4 in range(4):
        chunk(tc4)


def emit_proj_chunk(nc, pools, const, rep, tc4, xT, w_sb, qkT, v_sb,
                    ft_list=(0, 1, 2, 3), tt_list=None):
    _, big, work, outp, psA, psS, psO = pools
    cbqk = const["bqk_sb"]
    cbv = const["bv_sb"]
    cones = const["ones_sb"]
    if tt_list is None:
        tt_list = range(tc4 * 4, tc4 * 4 + 4)
    if True:
        ts = slice(tc4 * 512, (tc4 + 1) * 512)
        # head-pair 0 needs f-tiles 0 (Q) and 2 (K); pair 1 needs 1 and 3.
        # Emit pair-0's tiles and V first so attention hp=0 starts earlier.
        for ft in ft_list:
            ps = psA.tile([128, 512], F32, tag="mm", name=f"qkv{rep}_{tc4}_{ft}")
            for et in range(8):
                nc.tensor.matmul(
                    ps,
                    lhsT=w_sb[:, et, ft * 128:(ft + 1) * 128],
                    rhs=xT[:, et, ts],
                    start=(et == 0),
                    stop=(et == 7),
                )
            # bias add (per-partition) + cast to bf16, on DVE
            nc.vector.tensor_scalar_add(qkT[:, ft, ts], ps, cbqk[:, ft:ft + 1])
        for tt in tt_list:
            psv = psA.tile([128, FV], F32, tag="mm", name=f"v{rep}_{tt}")
            for et in range(8):
                nc.tensor.matmul(
                    psv,
                    lhsT=xT[:, et, tt * 128:(tt + 1) * 128],
                    rhs=w_sb[:, et, FQK:F],
                    start=(et == 0),
                    stop=False,
                )
            # bias via K=1 outer product: ones[t] x bv[f]
            nc.tensor.matmul(psv, lhsT=cones, rhs=cbv, start=False, stop=True)
            nc.vector.tensor_copy(
                v_sb[:, tt, :, 0:HD], psv.rearrange("p (h d) -> p h d", h=NH)
            )


def emit_attn_chunk(nc, pools, const, rep, hp, qc, qkT, v_sb, out_sb,
                    kt_lo=0, kt_hi=None, park_to=None, park_from=None):
    """Attention for head-pair hp, q-chunk qc, k-tiles [kt_lo, kt_hi].

    park_to: [128,2,4,65] SBUF tile — copy the partial O there and skip the
    epilogue (the chunk will be resumed later). park_from: merge a previously
    parked partial into the epilogue.
    """
    _, big, work, outp, psA, psS, psO = pools
    tri01 = const["tri01_sb"]
    if kt_hi is None:
        kt_hi = 4 * qc + 3
    if True:
        if True:
            psOs = [
                psO.tile([128, 4, HD + 1], F32, tag="O", name=f"O{rep}_{hp}_{qc}_{i}")
                for i in range(2)
            ]
            for kt in range(kt_lo, kt_hi + 1):
                q0 = max(qc * 512, kt * 128)
                qw = qc * 512 + 512 - q0
                pss = psS.tile([128, 2, 512], F32, tag="S",
                               name=f"S{rep}_{hp}_{qc}_{kt}")
                # the two heads use disjoint 64-row groups -> HW overlaps them
                for hh in range(2):
                    rs = hh * 64
                    nc.tensor.matmul(
                        pss[:, hh, 0:qw],
                        lhsT=qkT[rs:rs + 64, 2 + hp, kt * 128:(kt + 1) * 128],
                        rhs=qkT[rs:rs + 64, hp, q0:qc * 512 + 512],
                        start=True,
                        stop=True,
                    )
                pt = work.tile([128, 2, 512], BF16, tag="PT",
                               name=f"PT{rep}_{hp}_{qc}_{kt}")
                nc.scalar.activation(
                    out=pt[:, :, 0:qw],
                    in_=pss[:, :, 0:qw],
                    func=AF.Exp,
                    scale=SCALE,
                )
                diag = kt >= 4 * qc
                if diag:  # causal mask: zero lower triangle of first 128 cols
                    for hh in range(2):
                        nc.vector.tensor_mul(
                            pt[:, hh, 0:128], pt[:, hh, 0:128], tri01
                        )
                for hh in range(2):
                    h = hp * 2 + hh
                    # one PSUM accumulation group per bank: start on the first
                    # matmul into the bank, stop on the last; per-element
                    # has_written bits handle first-write vs accumulate.
                    for qs in range(max(kt, 4 * qc), 4 * qc + 4):
                        nc.tensor.matmul(
                            psOs[hh][:, qs - 4 * qc, :],
                            lhsT=pt[:, hh, qs * 128 - q0:qs * 128 - q0 + 128],
                            rhs=v_sb[:, kt, h, :],
                            start=(kt == kt_lo and qs == 4 * qc),
                            stop=(kt == kt_hi and qs == 4 * qc + 3),
                        )
            if park_to is not None:
                # stash partial O; the chunk resumes later with a fresh bank
                for hh in range(2):
                    nc.vector.tensor_copy(park_to[:, hh], psOs[hh])
                return
            # epilogue: divide by row sums, write to out_sb
            for hh in range(2):
                h = hp * 2 + hh
                if park_from is not None:
                    osum = work.tile([128, 4, HD + 1], F32, tag="osum",
                                     name=f"os{rep}_{hp}_{qc}_{hh}")
                    nc.vector.tensor_add(osum, psOs[hh], park_from[:, hh])
                    src = osum
                else:
                    src = psOs[hh]
                rsum = work.tile([128, 4, 1], F32, tag="rs",
                                 name=f"rs{rep}_{hp}_{qc}_{hh}")
                nc.vector.reciprocal(rsum, src[:, :, HD:HD + 1])
                for ql in range(4):
                    nc.vector.tensor_scalar_mul(
                        out_sb[:, 4 * qc + ql, h * HD:(h + 1) * HD],
                        src[:, ql, 0:HD],
                        rsum[:, ql, :],
                    )


def build_program(reps=1):
    nc = bacc.Bacc("TRN2", target_bir_lowering=False, debug=False, num_devices=8)
    xT_d = nc.dram_tensor("xT", [E, T], BF16, kind="ExternalInput").ap()
    wg_d = nc.dram_tensor("wg", [E, F], BF16, kind="ExternalInput").ap()
    bqk_d = nc.dram_tensor("bqk", [128, 4], F32, kind="ExternalInput").ap()
    bv_d = nc.dram_tensor("bv", [1, FV], BF16, kind="ExternalInput").ap()
    mask_d = nc.dram_tensor("tri01", [128, 128], BF16, kind="ExternalInput").ap()
    out_d = nc.dram_tensor("out", [T, FV], F32, kind="ExternalOutput").ap()

    with tile.TileContext(nc) as tcx, ExitStack() as ctx:
        constp = ctx.enter_context(tcx.tile_pool(name="constp", bufs=1))
        big = ctx.enter_context(tcx.tile_pool(name="big", bufs=1))
        work = ctx.enter_context(tcx.tile_pool(name="work", bufs=8))
        outp = ctx.enter_context(tcx.tile_pool(name="outp", bufs=1))
        psA = ctx.enter_context(tcx.tile_pool(name="psA", bufs=2, space="PSUM"))
        psS = ctx.enter_context(tcx.tile_pool(name="psS", bufs=2, space="PSUM"))
        psO = ctx.enter_context(tcx.tile_pool(name="psO", bufs=2, space="PSUM"))

        tri01_sb = constp.tile([128, 128], BF16)
        nc.sync.dma_start(out=tri01_sb, in_=mask_d)
        bqk_sb = constp.tile([128, 4], F32)
        nc.sync.dma_start(out=bqk_sb, in_=bqk_d)
        bv_sb = constp.tile([1, FV], BF16)
        nc.sync.dma_start(out=bv_sb, in_=bv_d)
        ones_sb = constp.tile([1, 128], BF16)
        nc.vector.memset(ones_sb, 1.0)
        const = {
            "tri01_sb": tri01_sb,
            "bqk_sb": bqk_sb,
            "bv_sb": bv_sb,
            "ones_sb": ones_sb,
        }

        pools = (const, big, work, outp, psA, psS, psO)
        dram = (xT_d, wg_d, bqk_d, bv_d, mask_d, out_d)
        for rep in range(reps):
            emit_body(nc, tcx, pools, dram, rep)

    nc.compile()
    return nc


def prep_core_inputs(x, W, b):
    """Host-side marshalling: slice per core, cast bf16, pre-transpose x."""
    tri01 = np.where(
        np.arange(128)[:, None] <= np.arange(128)[None, :], 1.0, 0.0
    ).astype(ml_dtypes.bfloat16)
    in_maps = []
    for c in range(8):
        bb, g = c // 4, c % 4
        cols = slice(g * 256, (g + 1) * 256)
        wg = np.concatenate(
            [W[:, cols], W[:, 1024:][:, cols], W[:, 2048:][:, cols]], axis=1
        )
        bq = b[cols]
        bk = b[1024:2048][cols]
        bv = b[2048:3072][cols]
        bqk = np.concatenate([bq, bk]).reshape(4, 128).T.copy()
        in_maps.append({
            "xT": np.ascontiguousarray(x[bb].T).astype(ml_dtypes.bfloat16),
            "wg": wg.astype(ml_dtypes.bfloat16),
            "bqk": bqk.astype(np.float32),
            "bv": bv.reshape(1, FV).astype(ml_dtypes.bfloat16),
            "tri01": tri01,
        })
    return in_maps


_nc_cache = {}


def get_program(reps=1):
    if reps not in _nc_cache:
        _nc_cache[reps] = build_program(reps)
    return _nc_cache[reps]


def kernel(x, W, b):
    x = np.asarray(x, dtype=np.float32)
    W = np.asarray(W, dtype=np.float32)
    b = np.asarray(b, dtype=np.float32)
    nc = get_program(1)
    in_maps = prep_core_inputs(x, W, b)
    res = run_bass_kernel_spmd(nc, in_maps, core_ids=list(range(8)))
    out = np.empty((2, T, 1024), dtype=np.float32)
    for c in range(8):
        bb, g = c // 4, c % 4
        out[bb, :, g * 256:(g + 1) * 256] = res.results[c]["out"]
    return out
